# revision 3
# baseline (speedup 1.0000x reference)
"""AttnDecoderRNN single-step kernel for 8 Trainium2 NeuronCores.

Sharding (tensor-parallel, per spec hint):
  - attention (tiny) replicated on every core
  - comb_w output-sharded (256 rows/core)  -> AllGather x (8KB)
  - w_ih/w_hh output-sharded (768 rows/core, r/z/n aligned) -> AllGather h_new
  - out_w vocab-row-sharded (6283 rows/core, core 7 overlaps core 6)
  - emb: only the one indexed row is needed; gathered on host (8KB)

The big matvecs run as fused multiply+reduce on the Vector engine
(weight rows on partitions, activation broadcast along free dim), which
needs no weight transpose and keeps the kernel HBM-bandwidth-bound.
"""

import numpy as np

H = 2048
V = 50257
L = 20
NC = 8
HS = H // NC          # 256 hidden rows per core
VS = 6283             # out_w rows per core (core 7 re-computes 7 rows)
VT = (VS + 127) // 128  # 50 vocab tiles per core
VPAD = VT * 128       # 6400
KC = 4096 // 128      # 32 contraction chunks for attn/comb matmuls

TRACE = False
LAST_RESULT = None

_NC_CACHE = {}


def _build():
    import concourse.bass as bass
    import concourse.bacc as bacc
    import concourse.tile as tile
    from concourse import mybir

    f32 = mybir.dt.float32
    AF = mybir.ActivationFunctionType
    OP = mybir.AluOpType
    RG = [list(range(NC))]

    nc = bacc.Bacc("TRN2", target_bir_lowering=False, debug=False, num_devices=NC)

    # ---- I/O ----
    ehcol = nc.dram_tensor("ehcol", [2 * H], f32, kind="ExternalInput")
    awt = nc.dram_tensor("awt", [2 * H, L], f32, kind="ExternalInput")
    ab = nc.dram_tensor("ab", [1, L], f32, kind="ExternalInput")
    enc = nc.dram_tensor("enc", [L, H], f32, kind="ExternalInput")
    cwt = nc.dram_tensor("cwt", [2 * H, HS], f32, kind="ExternalInput")
    cb = nc.dram_tensor("cb", [HS], f32, kind="ExternalInput")
    wg = {
        nm: nc.dram_tensor(nm, [HS, H], f32, kind="ExternalInput")
        for nm in ("wihr", "wihz", "wihn", "whhr", "whhz", "whhn")
    }
    bg = {
        nm: nc.dram_tensor(nm, [HS], f32, kind="ExternalInput")
        for nm in ("bihr", "bihz", "bihn", "bhhr", "bhhz", "bhhn")
    }
    h0row = nc.dram_tensor("h0row", [1, H], f32, kind="ExternalInput")
    h0k = nc.dram_tensor("h0k", [HS], f32, kind="ExternalInput")
    ow = nc.dram_tensor("ow", [VS, H], f32, kind="ExternalInput")
    ob = nc.dram_tensor("ob", [VPAD], f32, kind="ExternalInput")

    logits_o = nc.dram_tensor("logits", [128, VT], f32, kind="ExternalOutput")
    hnew_o = nc.dram_tensor("hnew", [1, H], f32, kind="ExternalOutput")
    attn_o = nc.dram_tensor("attn", [1, L], f32, kind="ExternalOutput")

    # collective bounce buffers (internal DRAM; collectives can't touch I/O)
    cc_in1 = nc.dram_tensor("cc_in1", [HS], f32)
    cc_out1 = nc.dram_tensor("cc_out1", [H], f32)
    cc_in2 = nc.dram_tensor("cc_in2", [HS], f32)
    cc_out2 = nc.dram_tensor("cc_out2", [H], f32)

    def bc(handle, n, cnt):
        # broadcast a contiguous DRAM range across n partitions
        a = handle.ap()
        return bass.AP(tensor=a.tensor, offset=a.offset, ap=[[0, n], [1, cnt]])

    with tile.TileContext(nc) as tc:
        with (
            tc.tile_pool(name="singles", bufs=1) as sg,
            tc.tile_pool(name="psum", bufs=1, space="PSUM") as ps,
            tc.tile_pool(name="gw", bufs=4) as gwp,
            tc.tile_pool(name="lw", bufs=8) as lwp,
            tc.tile_pool(name="scr", bufs=2) as scrp,
        ):
            # ---------- attention (replicated) ----------
            ec_sb = sg.tile([128, KC], f32)          # [emb; h0] column chunks
            nc.sync.dma_start(out=ec_sb, in_=ehcol.ap().rearrange("(c p) -> p c", p=128))
            awt_sb = sg.tile([128, KC, L], f32)      # attn_w.T chunks
            nc.sync.dma_start(
                out=awt_sb, in_=awt.ap().rearrange("(c p) j -> p c j", p=128)
            )
            ab_sb = sg.tile([1, L], f32)
            nc.sync.dma_start(out=ab_sb, in_=ab[:, :])
            enc_sb = sg.tile([L, H], f32)
            nc.sync.dma_start(out=enc_sb, in_=enc[:, :])

            pa = ps.tile([1, L], f32)
            for c in range(KC):
                nc.tensor.matmul(
                    pa[:, :], lhsT=ec_sb[:, c : c + 1], rhs=awt_sb[:, c, :],
                    start=(c == 0), stop=(c == KC - 1),
                )
            attn_sb = sg.tile([1, L], f32)
            nc.vector.tensor_add(attn_sb, pa[:, :], ab_sb)
            # softmax over the 20 logits (one partition)
            amax = sg.tile([1, 1], f32)
            nc.vector.reduce_max(amax, attn_sb, axis=mybir.AxisListType.X)
            negm = sg.tile([1, 1], f32)
            nc.vector.tensor_scalar_mul(negm, amax, -1.0)
            asum = sg.tile([1, 1], f32)
            nc.scalar.activation(attn_sb, attn_sb, AF.Exp, bias=negm, scale=1.0,
                                 accum_out=asum)
            rcp = sg.tile([1, 1], f32)
            nc.vector.reciprocal(rcp, asum)
            nc.vector.tensor_scalar_mul(attn_sb, attn_sb, rcp)
            nc.sync.dma_start(out=attn_o[:, :], in_=attn_sb)

            # attn_weights as a column for the context matmul
            acol = sg.tile([L, 1], f32)
            nc.sync.dma_start(out=acol, in_=attn_sb)

            # attn_applied, directly in column-chunk layout [128, 16]
            pctx = ps.tile([128, H // 128], f32)
            for i in range(H // 128):
                nc.tensor.matmul(
                    pctx[:, i : i + 1],
                    lhsT=enc_sb[:, i * 128 : (i + 1) * 128], rhs=acol,
                    start=True, stop=True,
                )
            ctxcol = sg.tile([128, H // 128], f32)
            nc.scalar.copy(ctxcol, pctx[:, :])

            # ---------- comb matvec -> x_k (TensorE, output-sharded) ----------
            cwt_sb = sg.tile([128, KC, HS], f32)
            nc.sync.dma_start(
                out=cwt_sb, in_=cwt.ap().rearrange("(c p) m -> p c m", p=128)
            )
            cb_sb = sg.tile([128, 2], f32)
            nc.sync.dma_start(out=cb_sb, in_=cb.ap().rearrange("(m p) -> p m", p=128))
            px = [ps.tile([128, 1], f32, name=f"px{m}", tag=f"px{m}") for m in range(2)]
            for m in range(2):
                for c in range(KC):
                    rhs = ec_sb[:, c : c + 1] if c < 16 else ctxcol[:, c - 16 : c - 15]
                    nc.tensor.matmul(
                        px[m][:, :],
                        lhsT=cwt_sb[:, c, m * 128 : (m + 1) * 128], rhs=rhs,
                        start=(c == 0), stop=(c == KC - 1),
                    )
            x_sb = sg.tile([128, 2], f32)
            for m in range(2):
                nc.scalar.activation(
                    x_sb[:, m : m + 1], px[m][:, :], AF.Relu,
                    bias=cb_sb[:, m : m + 1], scale=1.0,
                )
            nc.sync.dma_start(
                out=cc_in1.ap().rearrange("(m p) -> p m", p=128), in_=x_sb
            )

            # ---------- AllGather x ----------
            nc.gpsimd.collective_compute(
                "AllGather", OP.bypass, replica_groups=RG,
                ins=[cc_in1.ap()], outs=[cc_out1.ap()],
            )
            xb_sb = sg.tile([128, H], f32)
            nc.sync.dma_start(out=xb_sb, in_=bc(cc_out1, 128, H))
            h0b_sb = sg.tile([128, H], f32)
            nc.sync.dma_start(out=h0b_sb, in_=bc(h0row, 128, H))

            # ---------- GRU gates (VectorE fused mult+reduce) ----------
            bih_sb = sg.tile([128, 6], f32)
            bhh_sb = sg.tile([128, 6], f32)
            for g, nm in enumerate(("bihr", "bihz", "bihn")):
                nc.sync.dma_start(
                    out=bih_sb[:, 2 * g : 2 * g + 2],
                    in_=bg[nm].ap().rearrange("(m p) -> p m", p=128),
                )
            for g, nm in enumerate(("bhhr", "bhhz", "bhhn")):
                nc.sync.dma_start(
                    out=bhh_sb[:, 2 * g : 2 * g + 2],
                    in_=bg[nm].ap().rearrange("(m p) -> p m", p=128),
                )
            gi_sb = sg.tile([128, 6], f32)
            gh_sb = sg.tile([128, 6], f32)
            # gh first: depends only on h0 (no AllGather wait)
            for g, nm in enumerate(("whhr", "whhz", "whhn")):
                for hhalf in range(2):
                    j = 2 * g + hhalf
                    w_t = gwp.tile([128, H], f32, tag="gw")
                    nc.sync.dma_start(
                        out=w_t, in_=wg[nm][hhalf * 128 : (hhalf + 1) * 128, :]
                    )
                    s_t = scrp.tile([128, H], f32, tag="scr")
                    nc.vector.tensor_mul(s_t, w_t, h0b_sb)
                    nc.scalar.activation(s_t, s_t, AF.Identity,
                                         accum_out=gh_sb[:, j : j + 1])
            for g, nm in enumerate(("wihr", "wihz", "wihn")):
                for hhalf in range(2):
                    j = 2 * g + hhalf
                    w_t = gwp.tile([128, H], f32, tag="gw")
                    nc.sync.dma_start(
                        out=w_t, in_=wg[nm][hhalf * 128 : (hhalf + 1) * 128, :]
                    )
                    s_t = scrp.tile([128, H], f32, tag="scr")
                    nc.vector.tensor_mul(s_t, w_t, xb_sb)
                    nc.scalar.activation(s_t, s_t, AF.Identity,
                                         accum_out=gi_sb[:, j : j + 1])

            nc.vector.tensor_add(gi_sb, gi_sb, bih_sb)
            nc.vector.tensor_add(gh_sb, gh_sb, bhh_sb)

            # gates: r=sig(gi_r+gh_r) z=sig(gi_z+gh_z) n=tanh(gi_n + r*gh_n)
            rzt = sg.tile([128, 4], f32)
            nc.vector.tensor_add(rzt, gi_sb[:, 0:4], gh_sb[:, 0:4])
            rz = sg.tile([128, 4], f32)
            nc.scalar.activation(rz, rzt, AF.Sigmoid)
            nt = sg.tile([128, 2], f32)
            nc.vector.tensor_mul(nt, rz[:, 0:2], gh_sb[:, 4:6])
            nc.vector.tensor_add(nt, nt, gi_sb[:, 4:6])
            nn_sb = sg.tile([128, 2], f32)
            nc.scalar.activation(nn_sb, nt, AF.Tanh)
            h0k_sb = sg.tile([128, 2], f32)
            nc.sync.dma_start(
                out=h0k_sb, in_=h0k.ap().rearrange("(m p) -> p m", p=128)
            )
            # h' = n + z*(h0 - n)
            d_sb = sg.tile([128, 2], f32)
            nc.vector.tensor_sub(d_sb, h0k_sb, nn_sb)
            nc.vector.tensor_mul(d_sb, rz[:, 2:4], d_sb)
            hn_sb = sg.tile([128, 2], f32)
            nc.vector.tensor_add(hn_sb, nn_sb, d_sb)
            nc.sync.dma_start(
                out=cc_in2.ap().rearrange("(m p) -> p m", p=128), in_=hn_sb
            )

            # ---------- AllGather h_new ----------
            nc.gpsimd.collective_compute(
                "AllGather", OP.bypass, replica_groups=RG,
                ins=[cc_in2.ap()], outs=[cc_out2.ap()],
            )
            nc.sync.dma_start(
                out=hnew_o[:, :], in_=cc_out2.ap().rearrange("(a f) -> a f", a=1)
            )
            hb_sb = sg.tile([128, H], f32)
            nc.sync.dma_start(out=hb_sb, in_=bc(cc_out2, 128, H))

            # ---------- logits (VectorE fused mult+reduce, vocab-sharded) ----------
            ob_sb = sg.tile([128, VT], f32)
            nc.sync.dma_start(
                out=ob_sb, in_=ob.ap().rearrange("(t p) -> p t", p=128)
            )
            logit_sb = sg.tile([128, VT], f32)
            nc.vector.memset(logit_sb, 0.0)
            for t in range(VT):
                rows = min(128, VS - t * 128)
                w_t = lwp.tile([128, H], f32, tag="lw")
                nc.sync.dma_start(out=w_t[:rows, :], in_=ow[t * 128 : t * 128 + rows, :])
                s_t = scrp.tile([128, H], f32, tag="scr")
                nc.vector.tensor_mul(s_t[:rows, :], w_t[:rows, :], hb_sb[:rows, :])
                nc.scalar.activation(s_t[:rows, :], s_t[:rows, :], AF.Identity,
                                     accum_out=logit_sb[:rows, t : t + 1])
            nc.vector.tensor_add(logit_sb, logit_sb, ob_sb)
            nc.sync.dma_start(out=logits_o[:, :], in_=logit_sb)

    nc.compile()
    return nc


def _marshal(input_ids, hidden, encoder_outputs, emb, attn_w, attn_b,
             comb_w, comb_b, w_ih, w_hh, b_ih, b_hh, out_w, out_b):
    """Host-side sharding: returns one input map per core."""
    f = np.float32
    ii = int(np.asarray(input_ids).ravel()[0])
    emb_row = np.ascontiguousarray(np.asarray(emb)[ii], dtype=f)     # [H]
    h0 = np.ascontiguousarray(np.asarray(hidden, f).reshape(H))      # [H]
    ehcol = np.concatenate([emb_row, h0])                            # [2H]
    awt = np.ascontiguousarray(np.asarray(attn_w, f).T)              # [2H, L]
    ab = np.asarray(attn_b, f).reshape(1, L)
    enc = np.ascontiguousarray(np.asarray(encoder_outputs, f))       # [L, H]
    cw = np.asarray(comb_w, f)
    cbf = np.asarray(comb_b, f)
    wihf = np.asarray(w_ih, f)
    whhf = np.asarray(w_hh, f)
    bihf = np.asarray(b_ih, f)
    bhhf = np.asarray(b_hh, f)
    owf = np.asarray(out_w, f)
    obf = np.asarray(out_b, f)

    common = {
        "ehcol": ehcol, "awt": awt, "ab": ab, "enc": enc,
        "h0row": h0.reshape(1, H),
    }
    in_maps = []
    for k in range(NC):
        r0 = HS * k
        v0 = VS * k if k < NC - 1 else V - VS
        obk = np.zeros(VPAD, f)
        obk[:VS] = obf[v0 : v0 + VS]
        m = dict(common)
        m["cwt"] = np.ascontiguousarray(cw[r0 : r0 + HS].T)          # [2H, HS]
        m["cb"] = cbf[r0 : r0 + HS]
        m["wihr"] = wihf[r0 : r0 + HS]
        m["wihz"] = wihf[H + r0 : H + r0 + HS]
        m["wihn"] = wihf[2 * H + r0 : 2 * H + r0 + HS]
        m["whhr"] = whhf[r0 : r0 + HS]
        m["whhz"] = whhf[H + r0 : H + r0 + HS]
        m["whhn"] = whhf[2 * H + r0 : 2 * H + r0 + HS]
        m["bihr"] = bihf[r0 : r0 + HS]
        m["bihz"] = bihf[H + r0 : H + r0 + HS]
        m["bihn"] = bihf[2 * H + r0 : 2 * H + r0 + HS]
        m["bhhr"] = bhhf[r0 : r0 + HS]
        m["bhhz"] = bhhf[H + r0 : H + r0 + HS]
        m["bhhn"] = bhhf[2 * H + r0 : 2 * H + r0 + HS]
        m["h0k"] = h0[r0 : r0 + HS]
        m["ow"] = owf[v0 : v0 + VS]
        m["ob"] = obk
        in_maps.append(m)
    return in_maps


def kernel(**inputs):
    global LAST_RESULT
    from concourse.bass_utils import run_bass_kernel_spmd

    if "nc" not in _NC_CACHE:
        _NC_CACHE["nc"] = _build()
    nc = _NC_CACHE["nc"]

    in_maps = _marshal(**inputs)

    kwargs = {}
    if TRACE:
        import concourse.bass_utils as bu
        bu.upload_artifacts = lambda d: str(d)
        kwargs = dict(trace=True, trace_cores=[0])
    res = run_bass_kernel_spmd(nc, in_maps, core_ids=list(range(NC)), **kwargs)
    LAST_RESULT = res

    logits = np.empty((1, V), np.float32)
    for k in range(NC):
        v0 = VS * k if k < NC - 1 else V - VS
        arr = res.results[k]["logits"]          # [128, VT]
        logits[0, v0 : v0 + VS] = arr.T.reshape(-1)[:VS]
    hnew = res.results[0]["hnew"].reshape(1, 1, H).astype(np.float32)
    attn = res.results[0]["attn"].reshape(1, L).astype(np.float32)
    return logits, hnew, attn
